# revision 3
# baseline (speedup 1.0000x reference)
"""Multi-head attention (B=4, N=2048, E=768, H=12, D=64) on 8 TRN2 NeuronCores.

v3: 3-jb mode blocks at IC=512.
  - scores run as 4 concurrent 64x64 PE tiles, batched 3 key-blocks per
    64-mode stretch (sA/sB psum tiles [128, 1536] = 3 banks each)
  - exp in one big call per triple: ACT does head A exactly, DVE does head B
    via the Schraudolph int16 bit trick (bitcast to bf16)
  - PV (128-mode, M=65 ones-column) for triple i-1 + one proj fill chain run
    in the 128-mode stretch -> 2 mode switches per 3 jb instead of per jb
  - PSUM: sA 3 + sB 3 + oA 1 + oB 1 = 8 banks; fills borrow the sB slot
  - psum evacuations split: ACT takes v-proj/out-proj/k-proj copies and
    q-proj bias-adds (activation Identity + per-partition bias), DVE keeps
    its schraudolph + normalization
"""

import sys

if "/opt/trn_rl_repo" not in sys.path:
    sys.path.insert(0, "/opt/trn_rl_repo")

import numpy as np

B, N, E = 4, 2048, 768
H, D = 12, 64
HPC = 6
FQK = HPC * D
NCORES = 8
SCALE = D ** -0.5

LOG2E = 1.4426950408889634
A_S = 128.0 * LOG2E
B_S = 127.0 * 128.0 - 5.8

_CACHE = {}


def _build_bass():
    if "nc" in _CACHE:
        return _CACHE["nc"]

    from contextlib import ExitStack

    import concourse.bass as bass
    import concourse.tile as tile
    from concourse import bacc, mybir

    f32 = mybir.dt.float32
    bf16 = mybir.dt.bfloat16
    i16 = mybir.dt.int16
    Exp = mybir.ActivationFunctionType.Exp
    Identity = mybir.ActivationFunctionType.Identity
    mult = mybir.AluOpType.mult
    add = mybir.AluOpType.add

    nc = bacc.Bacc(
        "TRN2",
        target_bir_lowering=False,
        debug=False,
        num_devices=NCORES,
    )

    xT = nc.dram_tensor("xT", (E, N), bf16, kind="ExternalInput").ap()
    wqkT = nc.dram_tensor("wqkT", (E, 2 * FQK), bf16, kind="ExternalInput").ap()
    bq = nc.dram_tensor("bq", (FQK, 1), f32, kind="ExternalInput").ap()
    wvT = nc.dram_tensor("wvT", (E, FQK), bf16, kind="ExternalInput").ap()
    woT = nc.dram_tensor("woT", (FQK, E), bf16, kind="ExternalInput").ap()
    yp = nc.dram_tensor("yp", (N, E), f32, kind="ExternalOutput").ap()

    P = 128
    NCHUNK = 512
    IC = 512                 # query chunk per unit
    NPAIRS = HPC // 2
    NB = N // P              # 16 key blocks

    with ExitStack() as ctx:
        tc = ctx.enter_context(tile.TileContext(nc))

        wpool = ctx.enter_context(tc.tile_pool(name="w", bufs=1))
        wqk_t = []
        wv_t = []
        xe_t = []
        for t in range(6):
            wqk = wpool.tile([P, 2 * FQK], bf16, tag=f"wqk{t}", name=f"wqk{t}")
            nc.sync.dma_start(wqk[:], wqkT[t * P:(t + 1) * P, :])
            wqk_t.append(wqk)
            xe = wpool.tile([P, N], bf16, tag=f"xe{t}", name=f"xe{t}")
            nc.sync.dma_start(xe[:], xT[t * P:(t + 1) * P, :])
            xe_t.append(xe)
        for t in range(6):
            wv = wpool.tile([P, FQK], bf16, tag=f"wv{t}", name=f"wv{t}")
            nc.sync.dma_start(wv[:], wvT[t * P:(t + 1) * P, :])
            wv_t.append(wv)
        bqt = wpool.tile([P, 3], f32, tag="bq", name="bqt")
        for fb in range(3):
            nc.sync.dma_start(bqt[:, fb:fb + 1], bq[fb * P:(fb + 1) * P, :])

        qk_pool = ctx.enter_context(tc.tile_pool(name="qk", bufs=1))
        qkT_t = [
            qk_pool.tile([P, N], bf16, tag=f"qk{fb}", name=f"qkT{fb}")
            for fb in range(6)
        ]
        v_pool = ctx.enter_context(tc.tile_pool(name="v", bufs=1))
        v_t = [
            v_pool.tile([P, HPC * 65], bf16, tag=f"v{nb}", name=f"vv{nb}")
            for nb in range(NB)
        ]
        for nb in range(NB):
            nc.gpsimd.memset(v_t[nb][:], 1.0)

        wo_t = []
        for t in range(3):
            wo = wpool.tile([P, E], bf16, tag=f"wo{t}", name=f"wo{t}")
            nc.sync.dma_start(wo[:], woT[t * P:(t + 1) * P, :])
            wo_t.append(wo)

        psum = ctx.enter_context(tc.tile_pool(name="ps", bufs=1, space="PSUM"))

        pt_pool = ctx.enter_context(tc.tile_pool(name="pt", bufs=3))
        oT_pool = ctx.enter_context(tc.tile_pool(name="oT", bufs=1))
        oT_t = [
            oT_pool.tile([P, N], bf16, tag=f"oT{p}", name=f"oT{p}")
            for p in range(NPAIRS)
        ]
        nrm_pool = ctx.enter_context(tc.tile_pool(name="nrm", bufs=2))

        def proj_qk_chain(p, k, c4):
            """fb = q-block p or k-block 3+p; token chunk c4 of 512."""
            fb = (p, 3 + p)[k]
            n0 = c4 * NCHUNK

            def w():
                ps = psum.tile([P, NCHUNK], f32, tag="sB", name="ps_qk", bufs=2)
                for et in range(6):
                    nc.tensor.matmul(
                        ps[:],
                        lhsT=wqk_t[et][:, fb * P:(fb + 1) * P],
                        rhs=xe_t[et][:, n0:n0 + NCHUNK],
                        start=(et == 0),
                        stop=(et == 5),
                    )
                dst = qkT_t[fb][:, n0:n0 + NCHUNK]
                if fb < 3:
                    nc.scalar.activation(dst, ps[:], Identity,
                                         bias=bqt[:, fb:fb + 1])
                else:
                    nc.scalar.copy(dst, ps[:])

            return w

        def proj_v_chain(nb):
            def w():
                psv = psum.tile([P, FQK], f32, tag="sB", name="ps_v", bufs=2)
                for et in range(6):
                    nc.tensor.matmul(
                        psv[:],
                        lhsT=xe_t[et][:, nb * P:(nb + 1) * P],
                        rhs=wv_t[et][:],
                        start=(et == 0),
                        stop=(et == 5),
                    )
                v3t = v_t[nb].rearrange("p (h c) -> p h c", c=65)
                nc.scalar.activation(
                    v3t[:, :, 0:64],
                    psv.rearrange("p (h c) -> p h c", c=64),
                    Identity,
                )

            return w

        def out_proj_chain(ic, nb2, half):
            """Token block nb2 of query chunk ic; feature half of 384."""
            n0 = ic * IC + nb2 * P
            f0 = half * 384

            def w():
                psy = psum.tile([P, 384], f32, tag="sB", name="psy", bufs=2)
                for dt3 in range(3):
                    nc.tensor.matmul(
                        psy[:],
                        lhsT=oT_t[dt3][:, n0:n0 + P],
                        rhs=wo_t[dt3][:, f0:f0 + 384],
                        start=(dt3 == 0),
                        stop=(dt3 == 2),
                    )
                yt = nrm_pool.tile([P, 384], f32, tag="y", name="yt", bufs=3)
                nc.scalar.copy(yt[:], psy[:])
                nc.sync.dma_start(yp[n0:n0 + P, f0:f0 + 384], yt[:])

            return w

        def attention(p, ic, fill=None):
            """IC=512 queries; 2-jb pair blocks; exp per pair."""
            i0 = ic * IC
            qT = qkT_t[p]
            kT = qkT_t[3 + p]
            oA = psum.tile([65, IC], f32, tag="oA", name="oA")
            oB = psum.tile([65, IC], f32, tag="oB", name="oB")
            pend = []            # queued (ptA, ptB, jbs), PV lags 2 blocks
            fill = fill or {}

            def emit_pv(ptA, ptB, jbs):
                for kx, jb in enumerate(jbs):
                    us = slice(kx * 512, (kx + 1) * 512)
                    nc.tensor.matmul(
                        oA[:, 0:IC],
                        lhsT=v_t[jb][:, (2 * p) * 65:(2 * p) * 65 + 65],
                        rhs=ptA[:, us],
                        start=(jb == 0),
                        stop=(jb == NB - 1),
                    )
                    nc.tensor.matmul(
                        oB[:, 0:IC],
                        lhsT=v_t[jb][:, (2 * p + 1) * 65:(2 * p + 1) * 65 + 65],
                        rhs=ptB[:, us],
                        start=(jb == 0),
                        stop=(jb == NB - 1),
                    )

            for slot in range(NB // 2):
                jbs = [2 * slot, 2 * slot + 1]
                # --- 128-mode stretch first: PV lagging 2 blocks + fills
                if len(pend) >= 2:
                    emit_pv(*pend.pop(0))
                for w in fill.get(slot, ()):
                    w()
                sA = psum.tile([P, 1024], f32, tag="sA", name="sA")
                sB = psum.tile([P, 1024], f32, tag="sBx", name="sBx")
                # --- 64x64-tiled scores for the pair (4 tiles concurrent)
                for kx, jb in enumerate(jbs):
                    j0 = jb * P
                    us = slice(kx * 512, (kx + 1) * 512)
                    nc.tensor.matmul(
                        sA[0:64, us], lhsT=kT[0:64, j0:j0 + 64],
                        rhs=qT[0:64, i0:i0 + 512], start=True, stop=True,
                    )
                    nc.tensor.matmul(
                        sA[64:128, us], lhsT=kT[0:64, j0 + 64:j0 + 128],
                        rhs=qT[0:64, i0:i0 + 512], start=True, stop=True,
                    )
                    nc.tensor.matmul(
                        sB[0:64, us], lhsT=kT[64:128, j0:j0 + 64],
                        rhs=qT[64:128, i0:i0 + 512], start=True, stop=True,
                    )
                    nc.tensor.matmul(
                        sB[64:128, us], lhsT=kT[64:128, j0 + 64:j0 + 128],
                        rhs=qT[64:128, i0:i0 + 512], start=True, stop=True,
                    )
                # --- exp: one call per engine per pair
                ptA = pt_pool.tile([P, 1024], bf16, tag="ptA", name="ptA")
                nc.scalar.activation(ptA[:], sA[:], Exp)
                ptB_i = pt_pool.tile([P, 1024], i16, tag="ptB", name="ptB")
                nc.vector.tensor_scalar(
                    ptB_i[:], sB[:], A_S, B_S, mult, add,
                )
                pend.append((ptA, ptB_i.bitcast(bf16), jbs))
            for pv in pend:
                emit_pv(*pv)
            for w in fill.get(NB // 2, ()):
                w()

            # --- evacuate psum accumulators to SBUF fast (frees oA/oB for
            # the next unit), then normalize from the SBUF copy off-path
            for half, o_ps in ((0, oA), (1, oB)):
                oS = nrm_pool.tile([65, IC], f32, tag="oS", name="oS")
                nc.vector.tensor_copy(oS[:], o_ps[:])
                rs128 = nrm_pool.tile([P, IC // P], f32, tag="rs128",
                                      name="rs128")
                nc.sync.dma_start(rs128[:], oS[64:65, :])
                rcp = nrm_pool.tile([P, IC // P], f32, tag="rcp", name="rcp")
                nc.vector.reciprocal(rcp[:], rs128[:])
                rcpf = nrm_pool.tile([1, IC], f32, tag="rcpf", name="rcpf")
                nc.sync.dma_start(rcpf[:], rcp[:])
                rb = nrm_pool.tile([64, IC], f32, tag="rb", name="rb")
                nc.gpsimd.partition_broadcast(rb[:], rcpf[:])
                nc.vector.tensor_mul(
                    oT_t[p][half * 64:(half + 1) * 64, i0:i0 + IC],
                    oS[0:64, :],
                    rb[:],
                )

        # ---- schedule: pair-major units, fills spread into attention
        def unit_fills(u):
            """Fill chains for unit u (0..11): slot index -> chain list.

            Unit 0: v-proj just-in-time (pair slot t covers jbs of slot
            t+1; jbs 0,1 are done upfront).  Units 1-4: qk projections for
            pairs 1 and 2.  Units 9-11: out-projections for ic 0-2.
            """
            f = {}
            if u == 0:
                for slot in range(7):
                    f[slot] = [proj_v_chain(2 * slot + 2),
                               proj_v_chain(2 * slot + 3)]
                f[7] = [proj_qk_chain(0, 0, 1)]
            elif u in (1, 2):
                k0 = (u - 1) * 4
                for i, kc in enumerate(range(k0, k0 + 4)):
                    f[2 * i] = [proj_qk_chain(1, kc // 4, kc % 4)]
                f[1] = [proj_qk_chain(0, 0, u + 1)]
            elif u in (3, 4):
                k0 = (u - 3) * 4
                for i, kc in enumerate(range(k0, k0 + 4)):
                    f[2 * i] = [proj_qk_chain(2, kc // 4, kc % 4)]
            elif u in (9, 10, 11):
                ic_o = u - 9
                for i in range(8):
                    f[i] = [out_proj_chain(ic_o, i // 2, i % 2)]
            return f

        # HAM warmup: ~5us of dummy matmuls on the memset v tiles keeps the
        # PE busy through the DMA prelude and unthrottles the clock gate.
        wm = psum.tile([P, 384], f32, tag="sB", name="wm", bufs=2)
        for r in range(40):
            nc.tensor.matmul(
                wm[:], lhsT=v_t[0][:, 0:128], rhs=v_t[1][:, 0:384],
                start=True, stop=True,
            )
        for c4 in range(N // NCHUNK):
            proj_qk_chain(0, 1, c4)()
        proj_qk_chain(0, 0, 0)()
        proj_v_chain(0)()
        proj_v_chain(1)()
        u = 0
        for p in range(NPAIRS):
            for ic in range(N // IC):
                attention(p, ic, fill=unit_fills(u))
                u += 1
        for nb2 in range(4):
            for hf in range(2):
                out_proj_chain(3, nb2, hf)()

    nc.compile()
    _CACHE["nc"] = nc
    return nc


def _shard_inputs(x_q, w_qkv, b_qkv, w_out):
    import ml_dtypes

    mm_np = ml_dtypes.bfloat16

    def cmm(a):
        return np.ascontiguousarray(a.astype(mm_np))

    in_maps = []
    for c in range(NCORES):
        b = c // 2
        h0 = (c % 2) * HPC
        qs = slice(h0 * D, h0 * D + FQK)
        ks = slice(E + h0 * D, E + h0 * D + FQK)
        vs = slice(2 * E + h0 * D, 2 * E + h0 * D + FQK)
        wq = w_qkv[qs] * SCALE
        wk = w_qkv[ks]
        wv = w_qkv[vs]
        in_maps.append({
            "xT": cmm(x_q[b].T),
            "wqkT": cmm(np.concatenate([wq, wk], axis=0).T),
            "bq": np.ascontiguousarray(
                (b_qkv[qs] * SCALE).reshape(FQK, 1)),
            "wvT": cmm(wv.T),
            "woT": cmm(w_out[:, h0 * D:h0 * D + FQK].T),
        })
    return in_maps


def kernel(x_q, w_qkv, b_qkv, w_out, b_out, _trace=False, _tmpdir=None):
    x_q = np.asarray(x_q, dtype=np.float32)
    w_qkv = np.asarray(w_qkv, dtype=np.float32)
    b_qkv = np.asarray(b_qkv, dtype=np.float32)
    w_out = np.asarray(w_out, dtype=np.float32)
    b_out = np.asarray(b_out, dtype=np.float32)

    from concourse.bass_utils import run_bass_kernel_spmd

    nc = _build_bass()
    in_maps = _shard_inputs(x_q, w_qkv, b_qkv, w_out)
    res = run_bass_kernel_spmd(
        nc, in_maps, core_ids=list(range(NCORES)), trace=_trace, tmpdir=_tmpdir
    )
    _CACHE["last_result"] = res

    bv = b_qkv[2 * E:]
    b_eff = b_out + w_out @ bv
    y = np.empty((B, N, E), dtype=np.float32)
    for b in range(B):
        y[b] = res.results[2 * b]["yp"] + res.results[2 * b + 1]["yp"] + b_eff
    return y
